# revision 13
# baseline (speedup 1.0000x reference)
"""CascadedAttentionCell Trainium2 kernel.

Full shapes: inputs [64, 512, 1024] f32, prev_state [64, 1024] f32,
Wa [1024,1024], Ua [1024,1024], Va [1024,1], Ba [1,1024].
Output: context vector [64, 1024] f32.

Sharding: data-parallel over batch across 8 NeuronCores (8 batches per
core); weights replicated.

Host-side prep (numpy, <0.1% of FLOPs): WaSBa^T = (prev@Wa + Ba)^T,
Ua pre-packed to fp8e4 (x64 scale) in the [P, DC, OUT] DoubleRow
layout, Va^T in fp16, inputs cast to fp16, a [128,128] fp16 identity.

Per-core device plan (B=8 batches, T=512, D=1024, OUT=1024, P=128):
 - nat16 [P, TC, D] per batch from fp16 HBM on the scalar ring
   (t = 4p + c p-major layout, one 8KB descriptor per partition).
 - X^T: batches 0/1 via PE transposes + DVE scale-cast drains (PE is
   otherwise idle during the prologue); batches 2-7 via XBAR DMA
   transpose (SBUF->SBUF, sync ring only -- the scalar-ring XBAR
   corrupts data) + one DVE scale-cast each. Both paths fold the x16
   X scale into the fp8 cast.
 - main matmul fp8 DoubleRow (2 reduction rows/partition, 2x fp16
   throughput): st_ps[128,512] = sum over 4 k-pairs of
   Ua8[128,2,mc]^T @ xt8[128,2,512]; tanh on ACT removes the 1024x
   scale and adds the per-partition (WaS+Ba)^T bias in one pass.
 - z = Va^T @ S^T fp16 M=1 matmuls on PE; relu on DVE; softmax over T
   with exp accumulating its own sum on ACT (accum_out).
 - sm^T via tiny PE transposes; ctx[b] = sm^T @ nat16 fp16 matmuls
   (fp16 keeps the direct input->output path accurate), deferred one
   batch so the softmax latency hides under the next batch's mains.

Measured on trn2 (8 cores, axon): ~135 us HW exec, rel err ~1.4e-2
(vs 221 us for the fp16 baseline).
"""

import numpy as np
import ml_dtypes

import concourse.bass as bass
import concourse.tile as tile
import concourse.mybir as mybir
from concourse import bacc
from concourse.bass import ts
from concourse.bass_utils import run_bass_kernel_spmd
from concourse.masks import make_identity

f32 = mybir.dt.float32
f16 = mybir.dt.float16
f8 = mybir.dt.float8e4

N_CORES = 8
B = 8
T = 512
D = 1024
OUT = 1024
P = 128
DC = D // P
MC = OUT // P
TC = T // P

UA_SCALE = 64.0
X_SCALE = 16.0
INV_SCALE = 1.0 / (UA_SCALE * X_SCALE)

DR = mybir.MatmulPerfMode.DoubleRow

XBAR_BATCHES = tuple(range(2, B))  # X^T via XBAR (PE for startup batches)


def build_bass():
    nc = bacc.Bacc("TRN2", target_bir_lowering=False, debug=False,
                   num_devices=N_CORES)

    inp16 = nc.dram_tensor("inputs16", [B, T, D], f16, kind="ExternalInput").ap()
    Ua8d = nc.dram_tensor("ua8", [P, DC, OUT], f8, kind="ExternalInput").ap()
    Va16d = nc.dram_tensor("va16", [P, MC], f16, kind="ExternalInput").ap()
    wasd = nc.dram_tensor("wasbaT", [P, MC, B], f32, kind="ExternalInput").ap()
    identd = nc.dram_tensor("ident16", [P, P], f16, kind="ExternalInput").ap()
    out = nc.dram_tensor("out", [B, D], f32, kind="ExternalOutput").ap()

    with tile.TileContext(nc) as tc:
        with (
            tc.tile_pool(name="const", bufs=1) as const,
            tc.tile_pool(name="work", bufs=2) as work,
            tc.tile_pool(name="nat", bufs=B) as natp,
            tc.tile_pool(name="ps_big", bufs=4, space="PSUM") as ps_big,
            tc.tile_pool(name="ps_xt", bufs=1, space="PSUM") as ps_xt,
            tc.tile_pool(name="ps_small", bufs=2, space="PSUM") as ps_small,
            tc.tile_pool(name="ps_z", bufs=1, space="PSUM") as ps_z,
        ):
            # ---- small loads + Ua8 on the sync ring ----
            Va_sb = const.tile([P, MC], f16)
            nc.sync.dma_start(Va_sb[:], Va16d[:])
            WaSBaT_sb = const.tile([P, MC, B], f32)
            nc.sync.dma_start(WaSBaT_sb[:], wasd[:])
            Ua8 = const.tile([P, DC, OUT], f8)
            nc.sync.dma_start(Ua8[:], Ua8d[:])

            ident16 = const.tile([P, P], f16)
            nc.sync.dma_start(ident16[:], identd[:])

            nat16_tiles = {}
            xt_tiles = {}
            xt_casts = {}

            def load_input(b):
                # fp16 input, t = 4p + c: one 8KB descriptor per partition
                nat16 = natp.tile([P, TC, D], f16, tag="nat16")
                nat16_tiles[b] = nat16
                nc.scalar.dma_start(
                    nat16[:], inp16[b].rearrange("(p c) d -> p (c d)", p=P))

            def emit_xpose(b):
                # X^T: XBAR transpose (SBUF->SBUF, ring alternating by
                # batch), written CONTIGUOUS per tcI block (2KB descriptors
                # instead of 256B); the DVE scale-cast to fp8 permutes to
                # the [P, DC, T] matmul layout. Fallback: PE transposes.
                nat16 = nat16_tiles[b]
                xt = work.tile([P, DC, T], f8, tag="xt", bufs=3, name="xt")
                xt_tiles[b] = xt
                if b in XBAR_BATCHES:
                    xt16 = work.tile([P, DC, T], f16, tag="xt16", bufs=2,
                                     name="xt16")
                    for tcI in range(TC):
                        nc.sync.dma_start_transpose(
                            xt16[:, :, ts(tcI, P)], nat16[:, tcI, :])
                    nc.vector.tensor_scalar_mul(xt[:], xt16[:], X_SCALE)
                else:
                    for dc in range(DC):
                        xt_ps = ps_xt.tile([P, T], f16, tag="xtps")
                        for tcI in range(TC):
                            nc.tensor.transpose(xt_ps[:, ts(tcI, P)],
                                                nat16[:, tcI, ts(dc, P)],
                                                ident16[:])
                        nc.vector.tensor_scalar_mul(xt[:, dc, :], xt_ps[:],
                                                    X_SCALE)

            smT_sb = const.tile([P, TC, B], f16)
            sm16_tiles = {}

            def emit_smt_ctx(b):
                sm16 = sm16_tiles[b]
                smt_ps = ps_small.tile([P, TC, 2], f16, tag="psm")
                for tcI in range(TC):
                    nc.tensor.transpose(smt_ps[:, tcI, 0:1],
                                        sm16[:, ts(tcI, P)], ident16[:1, :1])
                nc.vector.tensor_copy(smT_sb[:, :, b], smt_ps[:, :, 0])

                nat16 = nat16_tiles[b]
                ctx_sb = work.tile([1, D], f32, tag="ctx")
                for n in range(2):
                    ctx_ps = ps_small.tile([1, T], f32, tag="psm")
                    for tcI in range(TC):
                        nc.tensor.matmul(ctx_ps[:], smT_sb[:, tcI, b:b + 1],
                                         nat16[:, tcI, ts(n, T)],
                                         start=(tcI == 0), stop=(tcI == TC - 1))
                    nc.vector.tensor_copy(ctx_sb[:, ts(n, T)], ctx_ps[:])
                nc.scalar.dma_start(out[b:b + 1, :], ctx_sb[:])

            def emit_softmax(b, z_ps):
                z16 = work.tile([1, T], f16, tag="z16")
                nc.vector.tensor_scalar_max(z16[:], z_ps[:], 0.0)
                negmax = work.tile([1, 1], f32, tag="nm")
                nc.vector.reduce_max(negmax[:], z16[:],
                                     axis=mybir.AxisListType.X, negate=True)
                esb = work.tile([1, T], f16, tag="esb")
                ssum = work.tile([1, 1], f32, tag="ss")
                nc.scalar.activation(esb[:], z16[:],
                                     mybir.ActivationFunctionType.Exp,
                                     bias=negmax[:], scale=1.0,
                                     accum_out=ssum[:])
                rsum = work.tile([1, 1], f32, tag="rs")
                nc.vector.reciprocal(rsum[:], ssum[:])
                sm16 = work.tile([1, T], f16, tag="sm16", bufs=3)
                sm16_tiles[b] = sm16
                nc.vector.tensor_scalar_mul(sm16[:], esb[:], rsum[:])

            # ---- prologue: batch 0 split across both rings for the
            # fastest possible start, batch 1 on the scalar ring ----
            nat0 = natp.tile([P, TC, D], f16, tag="nat16")
            nat16_tiles[0] = nat0
            src0 = inp16[0].rearrange("(p c) d -> p c d", p=P)
            nc.sync.dma_start(nat0[:, :TC // 2, :], src0[:, :TC // 2, :])
            nc.scalar.dma_start(nat0[:, TC // 2:, :], src0[:, TC // 2:, :])
            load_input(1)
            emit_xpose(0)
            emit_xpose(1)

            # ---------------- fully pipelined per-batch flow ----------------
            for b in range(B):
                if b + 2 < B:
                    load_input(b + 2)
                xt = xt_tiles[b]

                st = work.tile([P, MC, T], f16, tag="st", bufs=2)
                for mc in range(MC):
                    st_ps = ps_big.tile([P, T], f32, tag="stps")
                    for dc in range(0, DC, 2):
                        nc.tensor.matmul(st_ps[:], Ua8[:, dc:dc + 2, ts(mc, P)],
                                         xt[:, dc:dc + 2, :],
                                         start=(dc == 0), stop=(dc == DC - 2),
                                         perf_mode=DR)
                    nc.scalar.activation(st[:, mc, :], st_ps[:],
                                         mybir.ActivationFunctionType.Tanh,
                                         bias=WaSBaT_sb[:, mc, b:b + 1],
                                         scale=INV_SCALE)

                if b + 2 < B:
                    emit_xpose(b + 2)
                if b > 0:
                    emit_smt_ctx(b - 1)

                z_ps = ps_z.tile([1, T], f32, tag="zps")
                for mc in range(MC):
                    nc.tensor.matmul(z_ps[:], Va_sb[:, mc:mc + 1],
                                     st[:, mc, :],
                                     start=(mc == 0), stop=(mc == MC - 1))
                emit_softmax(b, z_ps)

            emit_smt_ctx(B - 1)

    nc.compile()
    return nc


_NC = None


def _get_nc():
    global _NC
    if _NC is None:
        _NC = build_bass()
    return _NC


def _prep(inputs, prev_state, Wa, Ua, Va, Ba):
    inputs = np.ascontiguousarray(inputs, dtype=np.float32)
    prev_state = np.ascontiguousarray(prev_state, dtype=np.float32)
    Wa = np.asarray(Wa, dtype=np.float32)
    Ua = np.asarray(Ua, dtype=np.float32)
    Va = np.asarray(Va, dtype=np.float32)
    Ba = np.asarray(Ba, dtype=np.float32)

    inp16 = inputs.astype(np.float16)
    # WaSBa^T per full batch: [OUT, B_total]
    wasba = (prev_state @ Wa + Ba).astype(np.float32)          # [B_total, OUT]
    wasbaT = np.ascontiguousarray(
        wasba.T.reshape(MC, P, -1).transpose(1, 0, 2))          # [P, MC, B_total]
    ua8 = np.ascontiguousarray(
        (Ua * UA_SCALE).reshape(DC, P, OUT).transpose(1, 0, 2)
    ).astype(ml_dtypes.float8_e4m3fn)                           # [P, DC, OUT]
    va16 = np.ascontiguousarray(
        Va.reshape(MC, P).T).astype(np.float16)                 # [P, MC]
    ident16 = np.eye(P, dtype=np.float16)
    return inp16, wasbaT, ua8, va16, ident16


def run(inputs, prev_state, Wa, Ua, Va, Ba, **spmd_kwargs):
    nc = _get_nc()
    inp16, wasbaT, ua8, va16, ident16 = _prep(inputs, prev_state, Wa, Ua, Va, Ba)
    in_maps = []
    for c in range(N_CORES):
        sl = slice(c * B, (c + 1) * B)
        in_maps.append({
            "inputs16": inp16[sl],
            "ua8": ua8,
            "va16": va16,
            "wasbaT": np.ascontiguousarray(wasbaT[:, :, sl]),
            "ident16": ident16,
        })
    return run_bass_kernel_spmd(nc, in_maps, core_ids=list(range(N_CORES)),
                                **spmd_kwargs)


def kernel(inputs, prev_state, Wa, Ua, Va, Ba):
    res = run(inputs, prev_state, Wa, Ua, Va, Ba)
    return np.concatenate([r["out"] for r in res.results], axis=0)


# revision 14
# speedup vs baseline: 1.1793x; 1.1793x over previous
"""CascadedAttentionCell Trainium2 kernel.

Full shapes: inputs [64, 512, 1024] f32, prev_state [64, 1024] f32,
Wa [1024,1024], Ua [1024,1024], Va [1024,1], Ba [1,1024].
Output: context vector [64, 1024] f32.

Sharding: data-parallel over batch across 8 NeuronCores (8 batches per
core); weights replicated.

Host-side prep (numpy, <0.1% of FLOPs): WaSBa^T = (prev@Wa + Ba)^T,
Ua pre-packed to fp8e4 (x64 scale) in the [P, DC, OUT] DoubleRow
layout, Va^T in fp16, inputs cast to fp16, a [128,128] fp16 identity.

Per-core device plan (B=8 batches, T=512, D=1024, OUT=1024, P=128):
 - nat16 [P, TC, D] per batch from fp16 HBM on the scalar ring
   (t = 4p + c p-major layout, one 8KB descriptor per partition).
 - X^T: batches 0/1 via PE transposes + DVE scale-cast drains (PE is
   otherwise idle during the prologue); batches 2-7 via XBAR DMA
   transpose (SBUF->SBUF, sync ring only -- the scalar-ring XBAR
   corrupts data) + one DVE scale-cast each. Both paths fold the x16
   X scale into the fp8 cast.
 - main matmul fp8 DoubleRow (2 reduction rows/partition, 2x fp16
   throughput): st_ps[128,512] = sum over 4 k-pairs of
   Ua8[128,2,mc]^T @ xt8[128,2,512]; tanh on ACT removes the 1024x
   scale and adds the per-partition (WaS+Ba)^T bias in one pass.
 - z = Va^T @ S^T fp16 M=1 matmuls on PE; relu on DVE; softmax over T
   with exp accumulating its own sum on ACT (accum_out).
 - sm^T via tiny PE transposes; ctx[b] = sm^T @ nat16 fp16 matmuls
   (fp16 keeps the direct input->output path accurate), deferred one
   batch so the softmax latency hides under the next batch's mains.

Measured on trn2 (8 cores, axon): ~135 us HW exec, rel err ~1.4e-2
(vs 221 us for the fp16 baseline).
"""

import numpy as np
import ml_dtypes

import concourse.bass as bass
import concourse.tile as tile
import concourse.mybir as mybir
from concourse import bacc
from concourse.bass import ts
from concourse.bass_utils import run_bass_kernel_spmd
from concourse.masks import make_identity

f32 = mybir.dt.float32
f16 = mybir.dt.float16
f8 = mybir.dt.float8e4

N_CORES = 8
B = 8
T = 512
D = 1024
OUT = 1024
P = 128
DC = D // P
MC = OUT // P
TC = T // P

UA_SCALE = 64.0
X_SCALE = 16.0
INV_SCALE = 1.0 / (UA_SCALE * X_SCALE)

DR = mybir.MatmulPerfMode.DoubleRow

XBAR_BATCHES = tuple(range(2, B))  # X^T via XBAR (PE for startup batches)


def build_bass():
    nc = bacc.Bacc("TRN2", target_bir_lowering=False, debug=False,
                   num_devices=N_CORES)

    inp16 = nc.dram_tensor("inputs16", [B, T, D], f16, kind="ExternalInput").ap()
    Ua8d = nc.dram_tensor("ua8", [P, DC, OUT], f8, kind="ExternalInput").ap()
    Va16d = nc.dram_tensor("va16", [P, MC], f16, kind="ExternalInput").ap()
    wasd = nc.dram_tensor("wasbaT", [P, MC, B], f32, kind="ExternalInput").ap()
    identd = nc.dram_tensor("ident16", [P, P], f16, kind="ExternalInput").ap()
    out = nc.dram_tensor("out", [B, D], f32, kind="ExternalOutput").ap()

    with tile.TileContext(nc) as tc:
        with (
            tc.tile_pool(name="const", bufs=1) as const,
            tc.tile_pool(name="work", bufs=2) as work,
            tc.tile_pool(name="nat", bufs=B) as natp,
            tc.tile_pool(name="ps_big", bufs=4, space="PSUM") as ps_big,
            tc.tile_pool(name="ps_xt", bufs=2, space="PSUM") as ps_xt,
            tc.tile_pool(name="ps_small", bufs=2, space="PSUM") as ps_small,
        ):
            # ---- small loads + Ua8 on the sync ring ----
            Va_sb = const.tile([P, MC], f16)
            nc.sync.dma_start(Va_sb[:], Va16d[:])
            WaSBaT_sb = const.tile([P, MC, B], f32)
            nc.sync.dma_start(WaSBaT_sb[:], wasd[:])
            Ua8 = const.tile([P, DC, OUT], f8)
            nc.sync.dma_start(Ua8[:], Ua8d[:])

            ident16 = const.tile([P, P], f16)
            nc.sync.dma_start(ident16[:], identd[:])

            nat16_tiles = {}
            xt_tiles = {}
            xt_casts = {}

            def load_input(b):
                # fp16 input, t = 4p + c: one 8KB descriptor per partition
                nat16 = natp.tile([P, TC, D], f16, tag="nat16")
                nat16_tiles[b] = nat16
                nc.scalar.dma_start(
                    nat16[:], inp16[b].rearrange("(p c) d -> p (c d)", p=P))

            def emit_xpose(b):
                # X^T: XBAR transpose (SBUF->SBUF, ring alternating by
                # batch), written CONTIGUOUS per tcI block (2KB descriptors
                # instead of 256B); the DVE scale-cast to fp8 permutes to
                # the [P, DC, T] matmul layout. Fallback: PE transposes.
                nat16 = nat16_tiles[b]
                xt = work.tile([P, DC, T], f8, tag="xt", bufs=3, name="xt")
                xt_tiles[b] = xt
                if b in XBAR_BATCHES:
                    xt16 = work.tile([P, DC, T], f16, tag="xt16", bufs=2,
                                     name="xt16")
                    for tcI in range(TC):
                        nc.sync.dma_start_transpose(
                            xt16[:, :, ts(tcI, P)], nat16[:, tcI, :])
                    nc.vector.tensor_scalar_mul(xt[:], xt16[:], X_SCALE)
                else:
                    for dc in range(DC):
                        xt_ps = ps_xt.tile([P, T], f16, tag="xtps")
                        for tcI in range(TC):
                            nc.tensor.transpose(xt_ps[:, ts(tcI, P)],
                                                nat16[:, tcI, ts(dc, P)],
                                                ident16[:])
                        nc.vector.tensor_scalar_mul(xt[:, dc, :], xt_ps[:],
                                                    X_SCALE)

            smT_sb = const.tile([P, TC, B], f16)
            sm16_tiles = {}

            def emit_smt_ctx(b):
                sm16 = sm16_tiles[b]
                smt_ps = ps_small.tile([P, TC, 2], f16, tag="psm")
                for tcI in range(TC):
                    nc.tensor.transpose(smt_ps[:, tcI, 0:1],
                                        sm16[:, ts(tcI, P)], ident16[:1, :1])
                nc.vector.tensor_copy(smT_sb[:, :, b], smt_ps[:, :, 0])

                nat16 = nat16_tiles[b]
                ctx_sb = work.tile([1, D], f32, tag="ctx")
                for n in range(2):
                    ctx_ps = ps_small.tile([1, T], f32, tag="psm")
                    for tcI in range(TC):
                        nc.tensor.matmul(ctx_ps[:], smT_sb[:, tcI, b:b + 1],
                                         nat16[:, tcI, ts(n, T)],
                                         start=(tcI == 0), stop=(tcI == TC - 1))
                    nc.vector.tensor_copy(ctx_sb[:, ts(n, T)], ctx_ps[:])
                nc.scalar.dma_start(out[b:b + 1, :], ctx_sb[:])

            def emit_softmax(b, z_ps):
                z16 = work.tile([1, T], f16, tag="z16")
                nc.vector.tensor_scalar_max(z16[:], z_ps[:], 0.0)
                negmax = work.tile([1, 1], f32, tag="nm")
                nc.vector.reduce_max(negmax[:], z16[:],
                                     axis=mybir.AxisListType.X, negate=True)
                esb = work.tile([1, T], f16, tag="esb")
                ssum = work.tile([1, 1], f32, tag="ss")
                nc.scalar.activation(esb[:], z16[:],
                                     mybir.ActivationFunctionType.Exp,
                                     bias=negmax[:], scale=1.0,
                                     accum_out=ssum[:])
                rsum = work.tile([1, 1], f32, tag="rs")
                nc.vector.reciprocal(rsum[:], ssum[:])
                sm16 = work.tile([1, T], f16, tag="sm16", bufs=3)
                sm16_tiles[b] = sm16
                nc.vector.tensor_scalar_mul(sm16[:], esb[:], rsum[:])

            # ---- prologue: first two batches in flight ----
            load_input(0)
            load_input(1)
            emit_xpose(0)
            emit_xpose(1)

            # ---------------- fully pipelined per-batch flow ----------------
            for b in range(B):
                if b + 2 < B:
                    load_input(b + 2)
                xt = xt_tiles[b]

                st = work.tile([P, MC, T], f16, tag="st", bufs=2)
                for mc in range(MC):
                    st_ps = ps_big.tile([P, T], f32, tag="stps")
                    for dc in range(0, DC, 2):
                        nc.tensor.matmul(st_ps[:], Ua8[:, dc:dc + 2, ts(mc, P)],
                                         xt[:, dc:dc + 2, :],
                                         start=(dc == 0), stop=(dc == DC - 2),
                                         perf_mode=DR)
                    nc.scalar.activation(st[:, mc, :], st_ps[:],
                                         mybir.ActivationFunctionType.Tanh,
                                         bias=WaSBaT_sb[:, mc, b:b + 1],
                                         scale=INV_SCALE)

                if b + 2 < B:
                    emit_xpose(b + 2)
                if b > 0:
                    emit_smt_ctx(b - 1)

                z_ps = ps_small.tile([1, T], f32, tag="psm")
                for mc in range(MC):
                    nc.tensor.matmul(z_ps[:], Va_sb[:, mc:mc + 1],
                                     st[:, mc, :],
                                     start=(mc == 0), stop=(mc == MC - 1))
                emit_softmax(b, z_ps)

            emit_smt_ctx(B - 1)

    nc.compile()
    return nc


_NC = None


def _get_nc():
    global _NC
    if _NC is None:
        _NC = build_bass()
    return _NC


def _prep(inputs, prev_state, Wa, Ua, Va, Ba):
    inputs = np.ascontiguousarray(inputs, dtype=np.float32)
    prev_state = np.ascontiguousarray(prev_state, dtype=np.float32)
    Wa = np.asarray(Wa, dtype=np.float32)
    Ua = np.asarray(Ua, dtype=np.float32)
    Va = np.asarray(Va, dtype=np.float32)
    Ba = np.asarray(Ba, dtype=np.float32)

    inp16 = inputs.astype(np.float16)
    # WaSBa^T per full batch: [OUT, B_total]
    wasba = (prev_state @ Wa + Ba).astype(np.float32)          # [B_total, OUT]
    wasbaT = np.ascontiguousarray(
        wasba.T.reshape(MC, P, -1).transpose(1, 0, 2))          # [P, MC, B_total]
    ua8 = np.ascontiguousarray(
        (Ua * UA_SCALE).reshape(DC, P, OUT).transpose(1, 0, 2)
    ).astype(ml_dtypes.float8_e4m3fn)                           # [P, DC, OUT]
    va16 = np.ascontiguousarray(
        Va.reshape(MC, P).T).astype(np.float16)                 # [P, MC]
    ident16 = np.eye(P, dtype=np.float16)
    return inp16, wasbaT, ua8, va16, ident16


def run(inputs, prev_state, Wa, Ua, Va, Ba, **spmd_kwargs):
    nc = _get_nc()
    inp16, wasbaT, ua8, va16, ident16 = _prep(inputs, prev_state, Wa, Ua, Va, Ba)
    in_maps = []
    for c in range(N_CORES):
        sl = slice(c * B, (c + 1) * B)
        in_maps.append({
            "inputs16": inp16[sl],
            "ua8": ua8,
            "va16": va16,
            "wasbaT": np.ascontiguousarray(wasbaT[:, :, sl]),
            "ident16": ident16,
        })
    return run_bass_kernel_spmd(nc, in_maps, core_ids=list(range(N_CORES)),
                                **spmd_kwargs)


def kernel(inputs, prev_state, Wa, Ua, Va, Ba):
    res = run(inputs, prev_state, Wa, Ua, Va, Ba)
    return np.concatenate([r["out"] for r in res.results], axis=0)


# revision 15
# speedup vs baseline: 1.1939x; 1.0124x over previous
"""CascadedAttentionCell Trainium2 kernel.

Full shapes: inputs [64, 512, 1024] f32, prev_state [64, 1024] f32,
Wa [1024,1024], Ua [1024,1024], Va [1024,1], Ba [1,1024].
Output: context vector [64, 1024] f32.

Sharding: data-parallel over batch across 8 NeuronCores (8 batches per
core); weights replicated.

Host-side prep (numpy, <0.1% of FLOPs): WaSBa^T = (prev@Wa + Ba)^T,
Ua pre-packed to fp8e4 (x64 scale) in the [P, DC, OUT] DoubleRow
layout, Va^T in fp16, inputs cast to fp16, a [128,128] fp16 identity.

Per-core device plan (B=8 batches, T=512, D=1024, OUT=1024, P=128):
 - nat16 [P, TC, D] per batch from fp16 HBM on the scalar ring
   (t = 4p + c p-major layout, one 8KB descriptor per partition).
 - X^T: batches 0/1 via PE transposes + DVE scale-cast drains (PE is
   otherwise idle during the prologue); batches 2-7 via XBAR DMA
   transpose (SBUF->SBUF, sync ring only -- the scalar-ring XBAR
   corrupts data) + one DVE scale-cast each. Both paths fold the x16
   X scale into the fp8 cast.
 - main matmul fp8 DoubleRow (2 reduction rows/partition, 2x fp16
   throughput): st_ps[128,512] = sum over 4 k-pairs of
   Ua8[128,2,mc]^T @ xt8[128,2,512]; tanh on ACT removes the 1024x
   scale and adds the per-partition (WaS+Ba)^T bias in one pass.
 - z = Va^T @ S^T fp16 M=1 matmuls on PE; relu on DVE; softmax over T
   with exp accumulating its own sum on ACT (accum_out).
 - sm^T via tiny PE transposes; ctx[b] = sm^T @ nat16 fp16 matmuls
   (fp16 keeps the direct input->output path accurate), deferred one
   batch so the softmax latency hides under the next batch's mains.

Measured on trn2 (8 cores, axon): ~135 us HW exec, rel err ~1.4e-2
(vs 221 us for the fp16 baseline).
"""

import numpy as np
import ml_dtypes

import concourse.bass as bass
import concourse.tile as tile
import concourse.mybir as mybir
from concourse import bacc
from concourse.bass import ts
from concourse.bass_utils import run_bass_kernel_spmd

f32 = mybir.dt.float32
f16 = mybir.dt.float16
f8 = mybir.dt.float8e4

N_CORES = 8
B = 8
T = 512
D = 1024
OUT = 1024
P = 128
DC = D // P
MC = OUT // P
TC = T // P

UA_SCALE = 64.0
X_SCALE = 16.0
INV_SCALE = 1.0 / (UA_SCALE * X_SCALE)

DR = mybir.MatmulPerfMode.DoubleRow

XBAR_BATCHES = tuple(range(2, B))  # X^T via XBAR (PE for startup batches)


def build_bass():
    nc = bacc.Bacc("TRN2", target_bir_lowering=False, debug=False,
                   num_devices=N_CORES)

    inp16 = nc.dram_tensor("inputs16", [B, T, D], f16, kind="ExternalInput").ap()
    Ua8d = nc.dram_tensor("ua8", [P, DC, OUT], f8, kind="ExternalInput").ap()
    Va16d = nc.dram_tensor("va16", [P, MC], f16, kind="ExternalInput").ap()
    wasd = nc.dram_tensor("wasbaT", [P, MC, B], f32, kind="ExternalInput").ap()
    identd = nc.dram_tensor("ident16", [P, P], f16, kind="ExternalInput").ap()
    out = nc.dram_tensor("out", [B, D], f32, kind="ExternalOutput").ap()

    with tile.TileContext(nc) as tc:
        with (
            tc.tile_pool(name="const", bufs=1) as const,
            tc.tile_pool(name="work", bufs=2) as work,
            tc.tile_pool(name="nat", bufs=B) as natp,
            tc.tile_pool(name="ps_big", bufs=4, space="PSUM") as ps_big,
            tc.tile_pool(name="ps_xt", bufs=2, space="PSUM") as ps_xt,
            tc.tile_pool(name="ps_small", bufs=2, space="PSUM") as ps_small,
        ):
            # ---- small loads + Ua8 on the sync ring ----
            Va_sb = const.tile([P, MC], f16)
            nc.sync.dma_start(Va_sb[:], Va16d[:])
            WaSBaT_sb = const.tile([P, MC, B], f32)
            nc.sync.dma_start(WaSBaT_sb[:], wasd[:])
            Ua8 = const.tile([P, DC, OUT], f8)
            nc.sync.dma_start(Ua8[:], Ua8d[:])

            ident16 = const.tile([P, P], f16)
            nc.sync.dma_start(ident16[:], identd[:])

            nat16_tiles = {}
            xt_tiles = {}

            def load_input(b):
                # fp16 input, t = 4p + c: one 8KB descriptor per partition
                nat16 = natp.tile([P, TC, D], f16, tag="nat16")
                nat16_tiles[b] = nat16
                nc.scalar.dma_start(
                    nat16[:], inp16[b].rearrange("(p c) d -> p (c d)", p=P))

            def emit_xpose(b):
                # X^T: XBAR transpose (SBUF->SBUF, ring alternating by
                # batch), written CONTIGUOUS per tcI block (2KB descriptors
                # instead of 256B); the DVE scale-cast to fp8 permutes to
                # the [P, DC, T] matmul layout. Fallback: PE transposes.
                nat16 = nat16_tiles[b]
                xt = work.tile([P, DC, T], f8, tag="xt", bufs=3, name="xt")
                xt_tiles[b] = xt
                if b in XBAR_BATCHES:
                    xt16 = work.tile([P, DC, T], f16, tag="xt16", bufs=2,
                                     name="xt16")
                    for tcI in range(TC):
                        nc.sync.dma_start_transpose(
                            xt16[:, :, ts(tcI, P)], nat16[:, tcI, :])
                    nc.vector.tensor_scalar_mul(xt[:], xt16[:], X_SCALE)
                else:
                    for dc in range(DC):
                        xt_ps = ps_xt.tile([P, T], f16, tag="xtps")
                        for tcI in range(TC):
                            nc.tensor.transpose(xt_ps[:, ts(tcI, P)],
                                                nat16[:, tcI, ts(dc, P)],
                                                ident16[:])
                        nc.vector.tensor_scalar_mul(xt[:, dc, :], xt_ps[:],
                                                    X_SCALE)

            smT_sb = const.tile([P, TC, B], f16)
            sm16_tiles = {}

            def emit_smt_ctx(b):
                sm16 = sm16_tiles[b]
                smt_ps = ps_small.tile([P, TC, 2], f16, tag="psm")
                for tcI in range(TC):
                    nc.tensor.transpose(smt_ps[:, tcI, 0:1],
                                        sm16[:, ts(tcI, P)], ident16[:1, :1])
                nc.vector.tensor_copy(smT_sb[:, :, b], smt_ps[:, :, 0])

                nat16 = nat16_tiles[b]
                ctx_sb = work.tile([1, D], f32, tag="ctx")
                for n in range(2):
                    ctx_ps = ps_small.tile([1, T], f32, tag="psm")
                    for tcI in range(TC):
                        nc.tensor.matmul(ctx_ps[:], smT_sb[:, tcI, b:b + 1],
                                         nat16[:, tcI, ts(n, T)],
                                         start=(tcI == 0), stop=(tcI == TC - 1))
                    nc.vector.tensor_copy(ctx_sb[:, ts(n, T)], ctx_ps[:])
                nc.scalar.dma_start(out[b:b + 1, :], ctx_sb[:])

            def emit_softmax(b, z_ps):
                z16 = work.tile([1, T], f16, tag="z16")
                nc.vector.tensor_scalar_max(z16[:], z_ps[:], 0.0)
                negmax = work.tile([1, 1], f32, tag="nm")
                nc.vector.reduce_max(negmax[:], z16[:],
                                     axis=mybir.AxisListType.X, negate=True)
                esb = work.tile([1, T], f16, tag="esb")
                ssum = work.tile([1, 1], f32, tag="ss")
                nc.scalar.activation(esb[:], z16[:],
                                     mybir.ActivationFunctionType.Exp,
                                     bias=negmax[:], scale=1.0,
                                     accum_out=ssum[:])
                rsum = work.tile([1, 1], f32, tag="rs")
                nc.vector.reciprocal(rsum[:], ssum[:])
                sm16 = work.tile([1, T], f16, tag="sm16", bufs=3)
                sm16_tiles[b] = sm16
                nc.vector.tensor_scalar_mul(sm16[:], esb[:], rsum[:])

            # ---- prologue: first two batches in flight ----
            load_input(0)
            load_input(1)
            emit_xpose(0)
            emit_xpose(1)

            # ---------------- fully pipelined per-batch flow ----------------
            for b in range(B):
                if b + 2 < B:
                    load_input(b + 2)
                xt = xt_tiles[b]

                st = work.tile([P, MC, T], f16, tag="st", bufs=2)
                for mc in range(MC):
                    st_ps = ps_big.tile([P, T], f32, tag="stps")
                    for dc in range(0, DC, 2):
                        nc.tensor.matmul(st_ps[:], Ua8[:, dc:dc + 2, ts(mc, P)],
                                         xt[:, dc:dc + 2, :],
                                         start=(dc == 0), stop=(dc == DC - 2),
                                         perf_mode=DR)
                    nc.scalar.activation(st[:, mc, :], st_ps[:],
                                         mybir.ActivationFunctionType.Tanh,
                                         bias=WaSBaT_sb[:, mc, b:b + 1],
                                         scale=INV_SCALE)

                if b + 2 < B:
                    emit_xpose(b + 2)
                if b > 0:
                    emit_smt_ctx(b - 1)

                z_ps = ps_small.tile([1, T], f32, tag="psm")
                for mc in range(MC):
                    nc.tensor.matmul(z_ps[:], Va_sb[:, mc:mc + 1],
                                     st[:, mc, :],
                                     start=(mc == 0), stop=(mc == MC - 1))
                emit_softmax(b, z_ps)

            emit_smt_ctx(B - 1)

    nc.compile()
    return nc


_NC = None


def _get_nc():
    global _NC
    if _NC is None:
        _NC = build_bass()
    return _NC


def _prep(inputs, prev_state, Wa, Ua, Va, Ba):
    inputs = np.ascontiguousarray(inputs, dtype=np.float32)
    prev_state = np.ascontiguousarray(prev_state, dtype=np.float32)
    Wa = np.asarray(Wa, dtype=np.float32)
    Ua = np.asarray(Ua, dtype=np.float32)
    Va = np.asarray(Va, dtype=np.float32)
    Ba = np.asarray(Ba, dtype=np.float32)

    inp16 = inputs.astype(np.float16)
    # WaSBa^T per full batch: [OUT, B_total]
    wasba = (prev_state @ Wa + Ba).astype(np.float32)          # [B_total, OUT]
    wasbaT = np.ascontiguousarray(
        wasba.T.reshape(MC, P, -1).transpose(1, 0, 2))          # [P, MC, B_total]
    ua8 = np.ascontiguousarray(
        (Ua * UA_SCALE).reshape(DC, P, OUT).transpose(1, 0, 2)
    ).astype(ml_dtypes.float8_e4m3fn)                           # [P, DC, OUT]
    va16 = np.ascontiguousarray(
        Va.reshape(MC, P).T).astype(np.float16)                 # [P, MC]
    ident16 = np.eye(P, dtype=np.float16)
    return inp16, wasbaT, ua8, va16, ident16


def run(inputs, prev_state, Wa, Ua, Va, Ba, **spmd_kwargs):
    nc = _get_nc()
    inp16, wasbaT, ua8, va16, ident16 = _prep(inputs, prev_state, Wa, Ua, Va, Ba)
    in_maps = []
    for c in range(N_CORES):
        sl = slice(c * B, (c + 1) * B)
        in_maps.append({
            "inputs16": inp16[sl],
            "ua8": ua8,
            "va16": va16,
            "wasbaT": np.ascontiguousarray(wasbaT[:, :, sl]),
            "ident16": ident16,
        })
    return run_bass_kernel_spmd(nc, in_maps, core_ids=list(range(N_CORES)),
                                **spmd_kwargs)


def kernel(inputs, prev_state, Wa, Ua, Va, Ba):
    res = run(inputs, prev_state, Wa, Ua, Va, Ba)
    return np.concatenate([r["out"] for r in res.results], axis=0)
